# revision 9
# baseline (speedup 1.0000x reference)
"""Trainium2 Bass kernel for nn_CrossDimEncoder (GNN message passing).

Strategy:
- Pure data parallel over batch: 2048/8 = 256 batch elements per NeuronCore.
- Feature-major layout on device: activations stored (D=128 partitions,
  token*batch free dim); every linear layer is then a plain matmul with the
  weight as the stationary operand.
- The tiny static adjacency tables are read on the host at build time and
  compiled into the matmul schedule: each gather-and-accumulate becomes a
  short sequence of PSUM-accumulating matmuls whose rhs is addressed
  directly at the source token's column block (no data movement). Degree
  scaling (1/cnt) is folded into pre-scaled weight variants; vertices and
  hexes are sorted by degree so weight reloads are rare.
- LayerNorm over the partition axis: column sums via a ones*(1/128) matmul
  (broadcast to all 128 partitions), rstd = Exp(-0.5*Ln(var+eps)) on the
  scalar engine, LN affine (g,b) folded into the following FFN weights.
- fp16 streams + weights (1 cycle/row on the PE vs 2 for fp32), fp32 PSUM.
- ACT table phasing: Ln/Exp phases and Gelu phases are kept separate so the
  activation-table reload (~1.3us) happens only 4x per block.
"""

import sys
import numpy as np

sys.path.insert(0, "/opt/trn_rl_repo")

# Problem constants (hardcoded per harness contract).
B = 2048
N_CORES = 8
BC = B // N_CORES          # 256 batch per core
TQ, NQ, EQ = 19, 54, 72    # hexes, vertices, edges
HIN, VIN, EIN, PIN = 16, 16, 8, 64
H, OUT, L = 128, 256, 4
WV = NQ * BC               # 13824
WH = TQ * BC               # 4864
WE = EQ * BC               # 18432
CH = 512                   # column chunk
F16 = "float16"

_CACHE = {}


def _chunks(width, ch=CH):
    out = []
    off = 0
    while off < width:
        out.append((off, min(ch, width - off)))
        off += ch
    return out


class WeightPack:
    """Packs all (K<=128, M<=128) stationary operands into one fp16 array
    (128, 128*ntiles) and all bias/scale vectors into one fp32 array."""

    def __init__(self):
        self.tiles = []   # list of (K, M, np.ndarray (K, M) f32)
        self.bias = []    # list of np.ndarray (P,) f32

    def add_w(self, w):
        w = np.asarray(w, np.float32)
        k, m = w.shape
        assert k <= 128 and m <= 128, (k, m)
        self.tiles.append((k, m, w))
        return len(self.tiles) - 1

    def add_b(self, b):
        b = np.asarray(b, np.float32).reshape(-1)
        assert b.shape[0] <= 128
        self.bias.append(b)
        return len(self.bias) - 1

    def finalize(self):
        nt = len(self.tiles)
        wp = np.zeros((128, 128 * nt), np.float16)
        for i, (k, m, w) in enumerate(self.tiles):
            wp[:k, 128 * i:128 * i + m] = w.astype(np.float16)
        nb = len(self.bias)
        bp = np.zeros((128, nb), np.float32)
        for j, b in enumerate(self.bias):
            bp[:b.shape[0], j] = b
        return wp, bp


def _prep(params, vertex_to_hex, hex_to_vertex, vertex_adj):
    """Host-side preprocessing: permutations, schedules, weight packing."""
    v2h = np.asarray(vertex_to_hex, np.int64)
    h2v = np.asarray(hex_to_vertex, np.int64)
    adj = np.asarray(vertex_adj, np.int64)

    # degree counts (clipped to >=1, matching reference's clip)
    cnt_v = np.maximum((adj >= 0).sum(1), 1)        # (54,) in 1..3
    cnt_h = np.maximum((h2v >= 0).sum(1), 1)        # (19,) in 1..6

    # sort vertices/hexes by degree so weight variants change rarely
    pv = np.argsort(cnt_v, kind="stable")           # new -> old
    ph = np.argsort(cnt_h, kind="stable")
    ipv = np.empty(NQ, np.int64); ipv[pv] = np.arange(NQ)
    iph = np.empty(TQ, np.int64); iph[ph] = np.arange(TQ)

    cnt_v_n = cnt_v[pv]
    cnt_h_n = cnt_h[ph]

    def remap(tbl, perm_rows, inv_cols):
        t = tbl[perm_rows]
        out = np.where(t >= 0, inv_cols[np.clip(t, 0, None)], -1)
        return out

    v2h_n = remap(v2h, pv, iph)      # (54,3) hex sources, new ids
    adj_n = remap(adj, pv, ipv)      # (54,3) vertex sources
    h2v_n = remap(h2v, ph, ipv)      # (19,6) vertex sources

    wp = WeightPack()
    g = lambda x: np.asarray(x, np.float32)

    blocks = []
    for p in params["blocks"]:
        infW, infb = g(p["inf_W"]), g(p["inf_b"])
        msgW, msgb = g(p["msg_W"]), g(p["msg_b"])
        updW, updb = g(p["upd_W"]), g(p["upd_b"])
        defW, defb = g(p["def_W"]), g(p["def_b"])
        vng, vnb = g(p["vn_g"]), g(p["vn_b"])
        hng, hnb = g(p["hn_g"]), g(p["hn_b"])
        vf1W, vf1b = g(p["vf1_W"]), g(p["vf1_b"])
        vf2W, vf2b = g(p["vf2_W"]), g(p["vf2_b"])
        hf1W, hf1b = g(p["hf1_W"]), g(p["hf1_b"])
        hf2W, hf2b = g(p["hf2_W"]), g(p["hf2_b"])

        d = {}
        d["inf"] = [wp.add_w(infW[s * H:(s + 1) * H]) for s in range(3)]
        d["inf_b"] = wp.add_b(infb)
        d["msg"] = {c: wp.add_w(msgW / c) for c in sorted(set(cnt_v_n.tolist()))}
        d["agg_b"] = {c: wp.add_b(3.0 * msgb / c)
                      for c in sorted(set(cnt_v_n.tolist()))}
        d["upd"] = [wp.add_w(updW[0:H]), wp.add_w(updW[H:2 * H])]
        d["upd_b"] = wp.add_b(updb)
        # fold v-LN affine into vf1:  vf1(y) with y = g*zhat + b
        vf1Wf = vng[:, None] * vf1W          # (128, 256)
        vf1bf = vnb @ vf1W + vf1b            # (256,)
        d["vf1"] = [wp.add_w(vf1Wf[:, 0:H]), wp.add_w(vf1Wf[:, H:2 * H])]
        d["vf1_b"] = [wp.add_b(vf1bf[0:H]), wp.add_b(vf1bf[H:2 * H])]
        d["vf2"] = [wp.add_w(vf2W[0:H]), wp.add_w(vf2W[H:2 * H])]
        d["v2_b"] = wp.add_b(vnb + vf2b)     # residual: v2 = g*zhat + (psum + vnb + vf2b)
        d["vn_g"] = wp.add_b(vng)
        d["def"] = {c: wp.add_w(defW / c) for c in sorted(set(cnt_h_n.tolist()))}
        d["def_b"] = wp.add_b(defb)
        hf1Wf = hng[:, None] * hf1W
        hf1bf = hnb @ hf1W + hf1b
        d["hf1"] = [wp.add_w(hf1Wf[:, 0:H]), wp.add_w(hf1Wf[:, H:2 * H])]
        d["hf1_b"] = [wp.add_b(hf1bf[0:H]), wp.add_b(hf1bf[H:2 * H])]
        d["hf2"] = [wp.add_w(hf2W[0:H]), wp.add_w(hf2W[H:2 * H])]
        d["h1_b"] = wp.add_b(hnb + hf2b)
        d["hn_g"] = wp.add_b(hng)
        blocks.append(d)

    head = {}
    hexW, hexb = g(params["hex_proj"][0]), g(params["hex_proj"][1])
    verW, verb = g(params["vertex_proj"][0]), g(params["vertex_proj"][1])
    edgW, edgb = g(params["edge_proj"][0]), g(params["edge_proj"][1])
    emlW, emlb = g(params["edge_mlp"][0]), g(params["edge_mlp"][1])
    p1W, p1b = g(params["player_mlp1"][0]), g(params["player_mlp1"][1])
    p2W, p2b = g(params["player_mlp2"][0]), g(params["player_mlp2"][1])
    o1W, o1b = g(params["out_mlp1"][0]), g(params["out_mlp1"][1])
    o2W, o2b = g(params["out_mlp2"][0]), g(params["out_mlp2"][1])

    head["hex"] = wp.add_w(hexW); head["hex_b"] = wp.add_b(hexb)
    head["ver"] = wp.add_w(verW); head["ver_b"] = wp.add_b(verb)
    # fused edge path: gelu(e_in @ (edgW@emlW) + (edgb@emlW + emlb))
    head["edge"] = wp.add_w(edgW @ emlW)                  # (8, 64)
    head["edge_b"] = wp.add_b(edgb @ emlW + emlb)         # (64,)
    head["I64"] = wp.add_w(np.eye(64, dtype=np.float32))
    head["I128"] = wp.add_w(np.eye(128, dtype=np.float32))
    head["p1"] = wp.add_w(p1W); head["p1_b"] = wp.add_b(p1b)
    head["p2"] = wp.add_w(p2W); head["p2_b"] = wp.add_b(p2b)
    # out_mlp1: fold pooling means into the K-rows
    o1 = o1W.copy()
    o1[0:128] /= TQ
    o1[128:256] /= NQ
    o1[256:320] /= EQ
    head["o1"] = [[wp.add_w(o1[r0:r1, m * 128:(m + 1) * 128])
                   for m in range(2)]
                  for (r0, r1) in [(0, 128), (128, 256), (256, 320), (320, 448)]]
    head["o1_b"] = [wp.add_b(o1b[0:128]), wp.add_b(o1b[128:256])]
    head["o2"] = [[wp.add_w(o2W[k * 128:(k + 1) * 128, m * 128:(m + 1) * 128])
                   for k in range(2)] for m in range(2)]
    head["o2_b"] = [wp.add_b(o2b[0:128]), wp.add_b(o2b[128:256])]
    head["Jones"] = wp.add_w(np.full((128, 128), 1.0 / 128, np.float32))
    head["Wzero"] = wp.add_w(np.zeros((128, 128), np.float32))
    head["zero_b"] = wp.add_b(np.zeros(128, np.float32))
    head["eps_b"] = wp.add_b(np.full(128, 1e-5, np.float32))

    # gather schedules: list per target block of (weight_tile_idx per block, src)
    sched = {
        "infl": [[(s, int(v2h_n[n, s])) for s in range(3) if v2h_n[n, s] >= 0]
                 for n in range(NQ)],
        "agg": [[int(adj_n[n, s]) for s in range(3) if adj_n[n, s] >= 0]
                for n in range(NQ)],
        "defl": [[int(h2v_n[t, s]) for s in range(6) if h2v_n[t, s] >= 0]
                 for t in range(TQ)],
        "cnt_v": cnt_v_n.tolist(),
        "cnt_h": cnt_h_n.tolist(),
    }

    wpack, bpack = wp.finalize()
    return wpack, bpack, blocks, head, sched, pv, ph


def _build(wcols, bcols, blocks, head, sched):
    """Builds the Bass/Tile program. Returns compiled nc."""
    import concourse.bass as bass
    import concourse.tile as tile
    from concourse import bacc, mybir

    f16 = mybir.dt.float16
    f32 = mybir.dt.float32
    AF = mybir.ActivationFunctionType
    OP = mybir.AluOpType

    # Restrict the activation-table chooser to two sets (indices preserved so
    # walrus's act_func_set_id remap stays valid). Without this the chooser
    # resolves Square/Identity/Exp and Ln to *different* sets and inserts a
    # ~1.3us ACT_TABLE_LOAD per LayerNorm chunk (~300 loads).
    import concourse.hw_specs as hw_specs
    _orig_gat = hw_specs.get_activation_tables
    _KEEP = ("natural_log_exp_and_others", "gelu_and_others")

    def _gat(arch):
        t = _orig_gat(arch)
        return {name: (funcs if name in _KEEP else set())
                for name, funcs in t.items()}

    nc = bacc.Bacc("TRN2", target_bir_lowering=False, debug=False)

    d_wp = nc.dram_tensor("wpack", [128, wcols], f16, kind="ExternalInput")
    d_bp = nc.dram_tensor("bpack", [128, bcols], f32, kind="ExternalInput")
    d_hex = nc.dram_tensor("hexT", [HIN, WH], f16, kind="ExternalInput")
    d_ver = nc.dram_tensor("vertT", [VIN, WV], f16, kind="ExternalInput")
    d_edg = nc.dram_tensor("edgeT", [EIN, WE], f16, kind="ExternalInput")
    d_ply = nc.dram_tensor("playerT", [PIN, BC], f16, kind="ExternalInput")
    d_out = nc.dram_tensor("outT", [OUT, BC], f32, kind="ExternalOutput")

    with tile.TileContext(nc) as tc, \
            tc.tile_pool(name="persist", bufs=1) as persist:
        wsb = persist.tile([128, wcols], f16, tag="wsb")
        bsb = persist.tile([128, bcols], f32, tag="bsb")
        nc.sync.dma_start(out=wsb, in_=d_wp.ap())
        nc.sync.dma_start(out=bsb, in_=d_bp.ap())

        vstream = persist.tile([128, WV], f16, tag="vstream")
        astream = persist.tile([128, WV], f16, tag="astream")
        hstream = persist.tile([128, WH], f16, tag="hstream")
        ep_sb = persist.tile([64, BC], f16, tag="ep_sb")
        pp_sb = persist.tile([128, BC], f16, tag="pp_sb")

        def W(i, k=128, m=128):
            return wsb[0:k, 128 * i:128 * i + m]

        def Bv(j, p=128):
            return bsb[0:p, j:j + 1]

        with (
            tc.tile_pool(name="ring", bufs=4) as ring,
            tc.tile_pool(name="ring2", bufs=4) as ring2,
            tc.tile_pool(name="lring", bufs=3) as lring,
            tc.tile_pool(name="psA", bufs=5, space="PSUM") as psA,
            tc.tile_pool(name="psB", bufs=2, space="PSUM") as psB,
            tc.tile_pool(name="psacc", bufs=1, space="PSUM") as psacc,
        ):
            # ---------------- edge path (independent) ----------------
            esum = psacc.tile([64, BC], f32)
            for ci, (off, sz) in enumerate(_chunks(WE)):
                et = ring.tile([EIN, CH], f16, tag="edgein")
                nc.sync.dma_start(out=et[:, 0:sz], in_=d_edg[:, off:off + sz])
                pe = psA.tile([64, CH], f32, tag="ps")
                nc.tensor.matmul(pe[:, 0:sz], W(head["edge"], k=EIN, m=64),
                                 et[:, 0:sz], start=True, stop=True)
                ge = ring.tile([64, CH], f16, tag="edgeg")
                nc.scalar.activation(ge[:, 0:sz], pe[:, 0:sz], AF.Gelu,
                                     bias=Bv(head["edge_b"], p=64))
                for hh in range(sz // BC):
                    nc.tensor.matmul(
                        esum, W(head["I64"], k=64, m=64),
                        ge[:, hh * BC:(hh + 1) * BC],
                        start=(ci == 0 and hh == 0),
                        stop=(ci == len(_chunks(WE)) - 1 and hh == sz // BC - 1))
            nc.scalar.copy(ep_sb, esum)

            # ---------------- player path ----------------
            pt = ring.tile([PIN, BC], f16, tag="ply")
            nc.sync.dma_start(out=pt, in_=d_ply.ap())
            pp1 = psA.tile([128, BC], f32, tag="ps")
            nc.tensor.matmul(pp1, W(head["p1"], k=PIN), pt, start=True, stop=True)
            s1 = ring.tile([128, BC], f16, tag="ply")
            nc.scalar.activation(s1, pp1, AF.Gelu, bias=Bv(head["p1_b"]))
            pp2 = psA.tile([128, BC], f32, tag="ps")
            nc.tensor.matmul(pp2, W(head["p2"]), s1, start=True, stop=True)
            nc.scalar.activation(pp_sb, pp2, AF.Gelu, bias=Bv(head["p2_b"]))

            # ---------------- projections ----------------
            for off, sz in _chunks(WH):
                xt = ring.tile([HIN, CH], f16, tag="projin")
                nc.sync.dma_start(out=xt[:, 0:sz], in_=d_hex[:, off:off + sz])
                pp = psA.tile([128, CH], f32, tag="ps")
                nc.tensor.matmul(pp[:, 0:sz], W(head["hex"], k=HIN),
                                 xt[:, 0:sz], start=True, stop=True)
                nc.scalar.activation(hstream[:, off:off + sz], pp[:, 0:sz],
                                     AF.Identity, bias=Bv(head["hex_b"]))
            for off, sz in _chunks(WV):
                xt = ring.tile([VIN, CH], f16, tag="projin")
                nc.sync.dma_start(out=xt[:, 0:sz], in_=d_ver[:, off:off + sz])
                pp = psA.tile([128, CH], f32, tag="ps")
                nc.tensor.matmul(pp[:, 0:sz], W(head["ver"], k=VIN),
                                 xt[:, 0:sz], start=True, stop=True)
                nc.scalar.activation(vstream[:, off:off + sz], pp[:, 0:sz],
                                     AF.Identity, bias=Bv(head["ver_b"]))

            # ---------------- blocks ----------------
            infl, aggs, defl = sched["infl"], sched["agg"], sched["defl"]
            cnt_v, cnt_h = sched["cnt_v"], sched["cnt_h"]

            def gather_into(ps, half, wtile_idx, srcs, src_buf, sz=BC):
                """accumulate sum_k W[wtile_idx[k]].T @ src_buf[:, srcs[k]] into
                psum half."""
                o0 = half * BC
                if not srcs:
                    nc.tensor.matmul(ps[:, o0:o0 + sz], W(head["Wzero"]),
                                     src_buf[:, 0:sz], start=True, stop=True)
                    return
                for k, (wi, t) in enumerate(zip(wtile_idx, srcs)):
                    nc.tensor.matmul(ps[:, o0:o0 + sz], W(wi),
                                     src_buf[:, t * BC:t * BC + sz],
                                     start=(k == 0), stop=(k == len(srcs) - 1))

            for li in range(L):
                blk = blocks[li]

                # --- inflate: v1 = v0 + inf(h0) + inf_b ---
                for pr in range(NQ // 2):
                    ps = psA.tile([128, CH], f32, tag="ps")
                    for half in (0, 1):
                        n = 2 * pr + half
                        sl = infl[n]
                        gather_into(ps, half, [blk["inf"][s] for s, _ in sl],
                                    [t for _, t in sl], hstream)
                    o = pr * CH
                    nc.vector.scalar_tensor_tensor(
                        out=vstream[:, o:o + CH], in0=ps, scalar=Bv(blk["inf_b"]),
                        in1=vstream[:, o:o + CH], op0=OP.add, op1=OP.add)

                # --- agg: fused message gather, weights pre-scaled 1/c ---
                for pr in range(NQ // 2):
                    ps = psA.tile([128, CH], f32, tag="ps")
                    for half in (0, 1):
                        n = 2 * pr + half
                        gather_into(ps, half, [blk["msg"][cnt_v[n]]] * len(aggs[n]),
                                    aggs[n], vstream)
                    n0 = 2 * pr
                    if cnt_v[n0] == cnt_v[n0 + 1]:
                        nc.scalar.activation(
                            astream[:, n0 * BC:(n0 + 2) * BC], ps,
                            AF.Identity, bias=Bv(blk["agg_b"][cnt_v[n0]]))
                    else:
                        for half in (0, 1):
                            n = n0 + half
                            nc.scalar.activation(
                                astream[:, n * BC:(n + 1) * BC],
                                ps[:, half * BC:(half + 1) * BC],
                                AF.Identity, bias=Bv(blk["agg_b"][cnt_v[n]]))

                # --- upd + x + LN_v (Ln/Exp table phase) ---
                for off, sz in _chunks(WV):
                    pm = psA.tile([128, CH], f32, tag="ps")
                    nc.tensor.matmul(pm[:, 0:sz], W(blk["upd"][0]),
                                     vstream[:, off:off + sz], start=True, stop=False)
                    nc.tensor.matmul(pm[:, 0:sz], W(blk["upd"][1]),
                                     astream[:, off:off + sz], start=False, stop=True)
                    # x = v1 + mp + upd_b   (in place over v1)
                    nc.vector.scalar_tensor_tensor(
                        out=vstream[:, off:off + sz], in0=pm[:, 0:sz],
                        scalar=Bv(blk["upd_b"]), in1=vstream[:, off:off + sz],
                        op0=OP.add, op1=OP.add)
                    # LN: mean broadcast
                    pP = psA.tile([128, CH], f32, tag="ps")
                    nc.tensor.matmul(pP[:, 0:sz], W(head["Jones"]),
                                     vstream[:, off:off + sz], start=True, stop=True)
                    nc.vector.tensor_sub(vstream[:, off:off + sz],
                                         vstream[:, off:off + sz], pP[:, 0:sz])
                    sq = ring2.tile([128, CH], f16, tag="sq")
                    nc.scalar.activation(sq[:, 0:sz], vstream[:, off:off + sz],
                                         AF.Square, bias=Bv(head["zero_b"]))
                    pQ = psA.tile([128, CH], f32, tag="ps")
                    nc.tensor.matmul(pQ[:, 0:sz], W(head["Jones"]),
                                     sq[:, 0:sz], start=True, stop=True)
                    lnt = lring.tile([128, CH], f32, tag="lnt")
                    nc.scalar.activation(lnt[:, 0:sz], pQ[:, 0:sz], AF.Ln,
                                         bias=Bv(head["eps_b"]))
                    rr = ring2.tile([128, CH], f16, tag="rr")
                    nc.scalar.activation(rr[:, 0:sz], lnt[:, 0:sz], AF.Exp,
                                         bias=Bv(head["zero_b"]), scale=-0.5)
                    nc.vector.tensor_mul(vstream[:, off:off + sz],
                                         vstream[:, off:off + sz], rr[:, 0:sz])

                # --- vf FFN (Gelu table phase): v2 = g*zhat + (vf2(...)+vnb+vf2b)
                for off, sz in _chunks(WV):
                    g1 = ring2.tile([128, 2 * CH], f16, tag="g1")
                    pf2 = psB.tile([128, CH], f32, tag="psf")
                    for m in (0, 1):
                        pf1 = psA.tile([128, CH], f32, tag="ps")
                        nc.tensor.matmul(pf1[:, 0:sz], W(blk["vf1"][m]),
                                         vstream[:, off:off + sz],
                                         start=True, stop=True)
                        nc.scalar.activation(g1[:, m * CH:m * CH + sz],
                                             pf1[:, 0:sz], AF.Gelu,
                                             bias=Bv(blk["vf1_b"][m]))
                        nc.tensor.matmul(pf2[:, 0:sz], W(blk["vf2"][m]),
                                         g1[:, m * CH:m * CH + sz],
                                         start=(m == 0), stop=(m == 1))
                    tres = ring2.tile([128, CH], f16, tag="tres")
                    nc.scalar.activation(tres[:, 0:sz], pf2[:, 0:sz],
                                         AF.Identity, bias=Bv(blk["v2_b"]))
                    nc.vector.scalar_tensor_tensor(
                        out=vstream[:, off:off + sz], in0=vstream[:, off:off + sz],
                        scalar=Bv(blk["vn_g"]), in1=tres[:, 0:sz],
                        op0=OP.mult, op1=OP.add)

                # --- deflate: xh = h0 + def(v2) + def_b  (in place on hstream)
                for pr in range((TQ + 1) // 2):
                    n0 = 2 * pr
                    nhalf = min(2, TQ - n0)
                    ps = psA.tile([128, CH], f32, tag="ps")
                    for half in range(nhalf):
                        t = n0 + half
                        c = cnt_h[t]
                        gather_into(ps, half, [blk["def"][c]] * len(defl[t]),
                                    defl[t], vstream)
                    o = pr * CH
                    sz = nhalf * BC
                    nc.vector.scalar_tensor_tensor(
                        out=hstream[:, o:o + sz], in0=ps[:, 0:sz],
                        scalar=Bv(blk["def_b"]), in1=hstream[:, o:o + sz],
                        op0=OP.add, op1=OP.add)

                # --- LN_h (Ln/Exp phase) ---
                for off, sz in _chunks(WH):
                    pP = psA.tile([128, CH], f32, tag="ps")
                    nc.tensor.matmul(pP[:, 0:sz], W(head["Jones"]),
                                     hstream[:, off:off + sz], start=True, stop=True)
                    nc.vector.tensor_sub(hstream[:, off:off + sz],
                                         hstream[:, off:off + sz], pP[:, 0:sz])
                    sq = ring2.tile([128, CH], f16, tag="sq")
                    nc.scalar.activation(sq[:, 0:sz], hstream[:, off:off + sz],
                                         AF.Square, bias=Bv(head["zero_b"]))
                    pQ = psA.tile([128, CH], f32, tag="ps")
                    nc.tensor.matmul(pQ[:, 0:sz], W(head["Jones"]),
                                     sq[:, 0:sz], start=True, stop=True)
                    lnt = lring.tile([128, CH], f32, tag="lnt")
                    nc.scalar.activation(lnt[:, 0:sz], pQ[:, 0:sz], AF.Ln,
                                         bias=Bv(head["eps_b"]))
                    rr = ring2.tile([128, CH], f16, tag="rr")
                    nc.scalar.activation(rr[:, 0:sz], lnt[:, 0:sz], AF.Exp,
                                         bias=Bv(head["zero_b"]), scale=-0.5)
                    nc.vector.tensor_mul(hstream[:, off:off + sz],
                                         hstream[:, off:off + sz], rr[:, 0:sz])

                # --- hf FFN (Gelu phase) ---
                for off, sz in _chunks(WH):
                    g1 = ring2.tile([128, 2 * CH], f16, tag="g1")
                    pf2 = psB.tile([128, CH], f32, tag="psf")
                    for m in (0, 1):
                        pf1 = psA.tile([128, CH], f32, tag="ps")
                        nc.tensor.matmul(pf1[:, 0:sz], W(blk["hf1"][m]),
                                         hstream[:, off:off + sz],
                                         start=True, stop=True)
                        nc.scalar.activation(g1[:, m * CH:m * CH + sz],
                                             pf1[:, 0:sz], AF.Gelu,
                                             bias=Bv(blk["hf1_b"][m]))
                        nc.tensor.matmul(pf2[:, 0:sz], W(blk["hf2"][m]),
                                         g1[:, m * CH:m * CH + sz],
                                         start=(m == 0), stop=(m == 1))
                    tres = ring2.tile([128, CH], f16, tag="tres")
                    nc.scalar.activation(tres[:, 0:sz], pf2[:, 0:sz],
                                         AF.Identity, bias=Bv(blk["h1_b"]))
                    nc.vector.scalar_tensor_tensor(
                        out=hstream[:, off:off + sz], in0=hstream[:, off:off + sz],
                        scalar=Bv(blk["hn_g"]), in1=tres[:, 0:sz],
                        op0=OP.mult, op1=OP.add)

            # ---------------- head: pooling + out MLP ----------------
            hsum = psB.tile([128, BC], f32, tag="psf")
            for t in range(TQ):
                nc.tensor.matmul(hsum, W(head["I128"]),
                                 hstream[:, t * BC:(t + 1) * BC],
                                 start=(t == 0), stop=(t == TQ - 1))
            hp = ring.tile([128, BC], f16, tag="pool")
            nc.scalar.copy(hp, hsum)
            vsum = psB.tile([128, BC], f32, tag="psf")
            for t in range(NQ):
                nc.tensor.matmul(vsum, W(head["I128"]),
                                 vstream[:, t * BC:(t + 1) * BC],
                                 start=(t == 0), stop=(t == NQ - 1))
            vp = ring.tile([128, BC], f16, tag="pool")
            nc.scalar.copy(vp, vsum)

            qt = ring.tile([128, 2 * BC], f16, tag="qt")
            for m in (0, 1):
                po1 = psA.tile([128, BC], f32, tag="ps")
                nc.tensor.matmul(po1, W(head["o1"][0][m]), hp, start=True, stop=False)
                nc.tensor.matmul(po1, W(head["o1"][1][m]), vp, start=False, stop=False)
                nc.tensor.matmul(po1, W(head["o1"][2][m], k=64), ep_sb,
                                 start=False, stop=False)
                nc.tensor.matmul(po1, W(head["o1"][3][m]), pp_sb,
                                 start=False, stop=True)
                nc.scalar.activation(qt[:, m * BC:(m + 1) * BC], po1, AF.Gelu,
                                     bias=Bv(head["o1_b"][m]))
            for m in (0, 1):
                po2 = psA.tile([128, BC], f32, tag="ps")
                nc.tensor.matmul(po2, W(head["o2"][m][0]), qt[:, 0:BC],
                                 start=True, stop=False)
                nc.tensor.matmul(po2, W(head["o2"][m][1]), qt[:, BC:2 * BC],
                                 start=False, stop=True)
                osb = ring.tile([128, BC], f32, tag="osb")
                nc.scalar.activation(osb, po2, AF.Identity,
                                     bias=Bv(head["o2_b"][m]))
                nc.sync.dma_start(out=d_out[m * 128:(m + 1) * 128, :], in_=osb)

    bacc.get_activation_tables = _gat
    try:
        nc.compile()
    finally:
        bacc.get_activation_tables = _orig_gat
    return nc


def _get_program(params, vertex_to_hex, hex_to_vertex, vertex_adj):
    key = (np.asarray(vertex_to_hex).tobytes(),
           np.asarray(hex_to_vertex).tobytes(),
           np.asarray(vertex_adj).tobytes())
    if key in _CACHE:
        return _CACHE[key]
    wpack, bpack, blocks, head, sched, pv, ph = _prep(
        params, vertex_to_hex, hex_to_vertex, vertex_adj)
    nc = _build(wpack.shape[1], bpack.shape[1], blocks, head, sched)
    _CACHE[key] = (nc, wpack, bpack, pv, ph)
    return _CACHE[key]


def kernel(hex_features, vertex_features, edge_features, player_features,
           params, vertex_to_hex, hex_to_vertex, vertex_adj):
    from concourse.bass_utils import run_bass_kernel_spmd

    nc, wpack, bpack, pv, ph = _get_program(
        params, vertex_to_hex, hex_to_vertex, vertex_adj)

    hexf = np.asarray(hex_features, np.float32)
    verf = np.asarray(vertex_features, np.float32)
    edgf = np.asarray(edge_features, np.float32)
    plyf = np.asarray(player_features, np.float32)

    in_maps = []
    for c in range(N_CORES):
        sl = slice(c * BC, (c + 1) * BC)
        hT = hexf[sl][:, ph, :].transpose(2, 1, 0).reshape(HIN, WH)
        vT = verf[sl][:, pv, :].transpose(2, 1, 0).reshape(VIN, WV)
        eT = edgf[sl].transpose(2, 1, 0).reshape(EIN, WE)
        pT = plyf[sl].T
        in_maps.append({
            "wpack": wpack,
            "bpack": bpack,
            "hexT": np.ascontiguousarray(hT, dtype=np.float16),
            "vertT": np.ascontiguousarray(vT, dtype=np.float16),
            "edgeT": np.ascontiguousarray(eT, dtype=np.float16),
            "playerT": np.ascontiguousarray(pT, dtype=np.float16),
        })

    res = run_bass_kernel_spmd(nc, in_maps, core_ids=list(range(N_CORES)))
    out = np.concatenate([r["outT"].T for r in res.results], axis=0)
    return out.astype(np.float32)


# revision 10
# speedup vs baseline: 1.0033x; 1.0033x over previous
"""Trainium2 Bass kernel for nn_CrossDimEncoder (GNN message passing).

Strategy:
- Pure data parallel over batch: 2048/8 = 256 batch elements per NeuronCore.
- Feature-major layout on device: activations stored (D=128 partitions,
  token*batch free dim); every linear layer is then a plain matmul with the
  weight as the stationary operand.
- The tiny static adjacency tables are read on the host at build time and
  compiled into the matmul schedule: each gather-and-accumulate becomes a
  short sequence of PSUM-accumulating matmuls whose rhs is addressed
  directly at the source token's column block (no data movement). Degree
  scaling (1/cnt) is folded into pre-scaled weight variants; vertices and
  hexes are sorted by degree so weight reloads are rare.
- LayerNorm over the partition axis: column sums via a ones*(1/128) matmul
  (broadcast to all 128 partitions), rstd = Exp(-0.5*Ln(var+eps)) on the
  scalar engine, LN affine (g,b) folded into the following FFN weights.
- fp16 streams + weights (1 cycle/row on the PE vs 2 for fp32), fp32 PSUM.
- ACT table phasing: Ln/Exp phases and Gelu phases are kept separate so the
  activation-table reload (~1.3us) happens only 4x per block.
"""

import sys
import numpy as np

sys.path.insert(0, "/opt/trn_rl_repo")

# Problem constants (hardcoded per harness contract).
B = 2048
N_CORES = 8
BC = B // N_CORES          # 256 batch per core
TQ, NQ, EQ = 19, 54, 72    # hexes, vertices, edges
HIN, VIN, EIN, PIN = 16, 16, 8, 64
H, OUT, L = 128, 256, 4
WV = NQ * BC               # 13824
WH = TQ * BC               # 4864
WE = EQ * BC               # 18432
CH = 512                   # column chunk
F16 = "float16"

_CACHE = {}


def _chunks(width, ch=CH):
    out = []
    off = 0
    while off < width:
        out.append((off, min(ch, width - off)))
        off += ch
    return out


class WeightPack:
    """Packs all (K<=128, M<=128) stationary operands into one fp16 array
    (128, 128*ntiles) and all bias/scale vectors into one fp32 array."""

    def __init__(self):
        self.tiles = []   # list of (K, M, np.ndarray (K, M) f32)
        self.bias = []    # list of np.ndarray (P,) f32

    def add_w(self, w):
        w = np.asarray(w, np.float32)
        k, m = w.shape
        assert k <= 128 and m <= 128, (k, m)
        self.tiles.append((k, m, w))
        return len(self.tiles) - 1

    def add_b(self, b):
        b = np.asarray(b, np.float32).reshape(-1)
        assert b.shape[0] <= 128
        self.bias.append(b)
        return len(self.bias) - 1

    def finalize(self):
        nt = len(self.tiles)
        wp = np.zeros((128, 128 * nt), np.float16)
        for i, (k, m, w) in enumerate(self.tiles):
            wp[:k, 128 * i:128 * i + m] = w.astype(np.float16)
        nb = len(self.bias)
        bp = np.zeros((128, nb), np.float32)
        for j, b in enumerate(self.bias):
            bp[:b.shape[0], j] = b
        return wp, bp


def _prep(params, vertex_to_hex, hex_to_vertex, vertex_adj):
    """Host-side preprocessing: permutations, schedules, weight packing."""
    v2h = np.asarray(vertex_to_hex, np.int64)
    h2v = np.asarray(hex_to_vertex, np.int64)
    adj = np.asarray(vertex_adj, np.int64)

    # degree counts (clipped to >=1, matching reference's clip)
    cnt_v = np.maximum((adj >= 0).sum(1), 1)        # (54,) in 1..3
    cnt_h = np.maximum((h2v >= 0).sum(1), 1)        # (19,) in 1..6

    # sort vertices/hexes by degree so weight variants change rarely
    pv = np.argsort(cnt_v, kind="stable")           # new -> old
    ph = np.argsort(cnt_h, kind="stable")
    ipv = np.empty(NQ, np.int64); ipv[pv] = np.arange(NQ)
    iph = np.empty(TQ, np.int64); iph[ph] = np.arange(TQ)

    cnt_v_n = cnt_v[pv]
    cnt_h_n = cnt_h[ph]

    def remap(tbl, perm_rows, inv_cols):
        t = tbl[perm_rows]
        out = np.where(t >= 0, inv_cols[np.clip(t, 0, None)], -1)
        return out

    v2h_n = remap(v2h, pv, iph)      # (54,3) hex sources, new ids
    adj_n = remap(adj, pv, ipv)      # (54,3) vertex sources
    h2v_n = remap(h2v, ph, ipv)      # (19,6) vertex sources

    wp = WeightPack()
    g = lambda x: np.asarray(x, np.float32)

    blocks = []
    for p in params["blocks"]:
        infW, infb = g(p["inf_W"]), g(p["inf_b"])
        msgW, msgb = g(p["msg_W"]), g(p["msg_b"])
        updW, updb = g(p["upd_W"]), g(p["upd_b"])
        defW, defb = g(p["def_W"]), g(p["def_b"])
        vng, vnb = g(p["vn_g"]), g(p["vn_b"])
        hng, hnb = g(p["hn_g"]), g(p["hn_b"])
        vf1W, vf1b = g(p["vf1_W"]), g(p["vf1_b"])
        vf2W, vf2b = g(p["vf2_W"]), g(p["vf2_b"])
        hf1W, hf1b = g(p["hf1_W"]), g(p["hf1_b"])
        hf2W, hf2b = g(p["hf2_W"]), g(p["hf2_b"])

        d = {}
        d["inf"] = [wp.add_w(infW[s * H:(s + 1) * H]) for s in range(3)]
        d["inf_b"] = wp.add_b(infb)
        d["msg"] = {c: wp.add_w(msgW / c) for c in sorted(set(cnt_v_n.tolist()))}
        d["agg_b"] = {c: wp.add_b(3.0 * msgb / c)
                      for c in sorted(set(cnt_v_n.tolist()))}
        d["upd"] = [wp.add_w(updW[0:H]), wp.add_w(updW[H:2 * H])]
        d["upd_b"] = wp.add_b(updb)
        # fold v-LN affine into vf1:  vf1(y) with y = g*zhat + b
        vf1Wf = vng[:, None] * vf1W          # (128, 256)
        vf1bf = vnb @ vf1W + vf1b            # (256,)
        d["vf1"] = [wp.add_w(vf1Wf[:, 0:H]), wp.add_w(vf1Wf[:, H:2 * H])]
        d["vf1_b"] = [wp.add_b(vf1bf[0:H]), wp.add_b(vf1bf[H:2 * H])]
        d["vf2"] = [wp.add_w(vf2W[0:H]), wp.add_w(vf2W[H:2 * H])]
        d["v2_b"] = wp.add_b(vnb + vf2b)     # residual: v2 = g*zhat + (psum + vnb + vf2b)
        d["vn_g"] = wp.add_b(vng)
        d["def"] = {c: wp.add_w(defW / c) for c in sorted(set(cnt_h_n.tolist()))}
        d["def_b"] = wp.add_b(defb)
        hf1Wf = hng[:, None] * hf1W
        hf1bf = hnb @ hf1W + hf1b
        d["hf1"] = [wp.add_w(hf1Wf[:, 0:H]), wp.add_w(hf1Wf[:, H:2 * H])]
        d["hf1_b"] = [wp.add_b(hf1bf[0:H]), wp.add_b(hf1bf[H:2 * H])]
        d["hf2"] = [wp.add_w(hf2W[0:H]), wp.add_w(hf2W[H:2 * H])]
        d["h1_b"] = wp.add_b(hnb + hf2b)
        d["hn_g"] = wp.add_b(hng)
        blocks.append(d)

    head = {}
    hexW, hexb = g(params["hex_proj"][0]), g(params["hex_proj"][1])
    verW, verb = g(params["vertex_proj"][0]), g(params["vertex_proj"][1])
    edgW, edgb = g(params["edge_proj"][0]), g(params["edge_proj"][1])
    emlW, emlb = g(params["edge_mlp"][0]), g(params["edge_mlp"][1])
    p1W, p1b = g(params["player_mlp1"][0]), g(params["player_mlp1"][1])
    p2W, p2b = g(params["player_mlp2"][0]), g(params["player_mlp2"][1])
    o1W, o1b = g(params["out_mlp1"][0]), g(params["out_mlp1"][1])
    o2W, o2b = g(params["out_mlp2"][0]), g(params["out_mlp2"][1])

    head["hex"] = wp.add_w(hexW); head["hex_b"] = wp.add_b(hexb)
    head["ver"] = wp.add_w(verW); head["ver_b"] = wp.add_b(verb)
    # fused edge path: gelu(e_in @ (edgW@emlW) + (edgb@emlW + emlb))
    head["edge"] = wp.add_w(edgW @ emlW)                  # (8, 64)
    head["edge_b"] = wp.add_b(edgb @ emlW + emlb)         # (64,)
    head["I64"] = wp.add_w(np.eye(64, dtype=np.float32))
    head["I128"] = wp.add_w(np.eye(128, dtype=np.float32))
    head["p1"] = wp.add_w(p1W); head["p1_b"] = wp.add_b(p1b)
    head["p2"] = wp.add_w(p2W); head["p2_b"] = wp.add_b(p2b)
    # out_mlp1: fold pooling means into the K-rows
    o1 = o1W.copy()
    o1[0:128] /= TQ
    o1[128:256] /= NQ
    o1[256:320] /= EQ
    head["o1"] = [[wp.add_w(o1[r0:r1, m * 128:(m + 1) * 128])
                   for m in range(2)]
                  for (r0, r1) in [(0, 128), (128, 256), (256, 320), (320, 448)]]
    head["o1_b"] = [wp.add_b(o1b[0:128]), wp.add_b(o1b[128:256])]
    head["o2"] = [[wp.add_w(o2W[k * 128:(k + 1) * 128, m * 128:(m + 1) * 128])
                   for k in range(2)] for m in range(2)]
    head["o2_b"] = [wp.add_b(o2b[0:128]), wp.add_b(o2b[128:256])]
    head["Jones"] = wp.add_w(np.full((128, 128), 1.0 / 128, np.float32))
    head["Wzero"] = wp.add_w(np.zeros((128, 128), np.float32))
    head["zero_b"] = wp.add_b(np.zeros(128, np.float32))
    head["eps_b"] = wp.add_b(np.full(128, 1e-5, np.float32))

    # gather schedules: list per target block of (weight_tile_idx per block, src)
    sched = {
        "infl": [[(s, int(v2h_n[n, s])) for s in range(3) if v2h_n[n, s] >= 0]
                 for n in range(NQ)],
        "agg": [[int(adj_n[n, s]) for s in range(3) if adj_n[n, s] >= 0]
                for n in range(NQ)],
        "defl": [[int(h2v_n[t, s]) for s in range(6) if h2v_n[t, s] >= 0]
                 for t in range(TQ)],
        "cnt_v": cnt_v_n.tolist(),
        "cnt_h": cnt_h_n.tolist(),
    }

    wpack, bpack = wp.finalize()
    return wpack, bpack, blocks, head, sched, pv, ph


def _build(wcols, bcols, blocks, head, sched):
    """Builds the Bass/Tile program. Returns compiled nc."""
    import concourse.bass as bass
    import concourse.tile as tile
    from concourse import bacc, mybir

    f16 = mybir.dt.float16
    f32 = mybir.dt.float32
    AF = mybir.ActivationFunctionType
    OP = mybir.AluOpType

    # Restrict the activation-table chooser to two sets (indices preserved so
    # walrus's act_func_set_id remap stays valid). Without this the chooser
    # resolves Square/Identity/Exp and Ln to *different* sets and inserts a
    # ~1.3us ACT_TABLE_LOAD per LayerNorm chunk (~300 loads).
    import concourse.hw_specs as hw_specs
    _orig_gat = hw_specs.get_activation_tables
    _KEEP = ("natural_log_exp_and_others", "gelu_and_others")

    def _gat(arch):
        t = _orig_gat(arch)
        return {name: (funcs if name in _KEEP else set())
                for name, funcs in t.items()}

    nc = bacc.Bacc("TRN2", target_bir_lowering=False, debug=False)

    d_wp = nc.dram_tensor("wpack", [128, wcols], f16, kind="ExternalInput")
    d_bp = nc.dram_tensor("bpack", [128, bcols], f32, kind="ExternalInput")
    d_hex = nc.dram_tensor("hexT", [HIN, WH], f16, kind="ExternalInput")
    d_ver = nc.dram_tensor("vertT", [VIN, WV], f16, kind="ExternalInput")
    d_edg = nc.dram_tensor("edgeT", [EIN, WE], f16, kind="ExternalInput")
    d_ply = nc.dram_tensor("playerT", [PIN, BC], f16, kind="ExternalInput")
    d_out = nc.dram_tensor("outT", [OUT, BC], f32, kind="ExternalOutput")

    with tile.TileContext(nc) as tc, \
            tc.tile_pool(name="persist", bufs=1) as persist:
        wsb = persist.tile([128, wcols], f16, tag="wsb")
        bsb = persist.tile([128, bcols], f32, tag="bsb")
        nc.sync.dma_start(out=wsb, in_=d_wp.ap())
        nc.sync.dma_start(out=bsb, in_=d_bp.ap())

        vstream = persist.tile([128, WV], f16, tag="vstream")
        astream = persist.tile([128, WV], f16, tag="astream")
        hstream = persist.tile([128, WH], f16, tag="hstream")
        ep_sb = persist.tile([64, BC], f16, tag="ep_sb")
        pp_sb = persist.tile([128, BC], f16, tag="pp_sb")

        def W(i, k=128, m=128):
            return wsb[0:k, 128 * i:128 * i + m]

        def Bv(j, p=128):
            return bsb[0:p, j:j + 1]

        with (
            tc.tile_pool(name="ring", bufs=4) as ring,
            tc.tile_pool(name="ring2", bufs=4) as ring2,
            tc.tile_pool(name="lring", bufs=3) as lring,
            tc.tile_pool(name="psA", bufs=6, space="PSUM") as psA,
            tc.tile_pool(name="psB", bufs=2, space="PSUM") as psB,
        ):
            # ---------------- edge path (independent) ----------------
            esum = psB.tile([64, BC], f32, tag="psf")
            for ci, (off, sz) in enumerate(_chunks(WE)):
                et = ring.tile([EIN, CH], f16, tag="edgein")
                nc.sync.dma_start(out=et[:, 0:sz], in_=d_edg[:, off:off + sz])
                pe = psA.tile([64, CH], f32, tag="ps")
                nc.tensor.matmul(pe[:, 0:sz], W(head["edge"], k=EIN, m=64),
                                 et[:, 0:sz], start=True, stop=True)
                ge = ring.tile([64, CH], f16, tag="edgeg")
                nc.scalar.activation(ge[:, 0:sz], pe[:, 0:sz], AF.Gelu,
                                     bias=Bv(head["edge_b"], p=64))
                for hh in range(sz // BC):
                    nc.tensor.matmul(
                        esum, W(head["I64"], k=64, m=64),
                        ge[:, hh * BC:(hh + 1) * BC],
                        start=(ci == 0 and hh == 0),
                        stop=(ci == len(_chunks(WE)) - 1 and hh == sz // BC - 1))
            nc.scalar.copy(ep_sb, esum)

            # ---------------- player path ----------------
            pt = ring.tile([PIN, BC], f16, tag="ply")
            nc.sync.dma_start(out=pt, in_=d_ply.ap())
            pp1 = psA.tile([128, BC], f32, tag="ps")
            nc.tensor.matmul(pp1, W(head["p1"], k=PIN), pt, start=True, stop=True)
            s1 = ring.tile([128, BC], f16, tag="ply")
            nc.scalar.activation(s1, pp1, AF.Gelu, bias=Bv(head["p1_b"]))
            pp2 = psA.tile([128, BC], f32, tag="ps")
            nc.tensor.matmul(pp2, W(head["p2"]), s1, start=True, stop=True)
            nc.scalar.activation(pp_sb, pp2, AF.Gelu, bias=Bv(head["p2_b"]))

            # ---------------- projections ----------------
            for off, sz in _chunks(WH):
                xt = ring.tile([HIN, CH], f16, tag="projin")
                nc.sync.dma_start(out=xt[:, 0:sz], in_=d_hex[:, off:off + sz])
                pp = psA.tile([128, CH], f32, tag="ps")
                nc.tensor.matmul(pp[:, 0:sz], W(head["hex"], k=HIN),
                                 xt[:, 0:sz], start=True, stop=True)
                nc.scalar.activation(hstream[:, off:off + sz], pp[:, 0:sz],
                                     AF.Identity, bias=Bv(head["hex_b"]))
            for off, sz in _chunks(WV):
                xt = ring.tile([VIN, CH], f16, tag="projin")
                nc.sync.dma_start(out=xt[:, 0:sz], in_=d_ver[:, off:off + sz])
                pp = psA.tile([128, CH], f32, tag="ps")
                nc.tensor.matmul(pp[:, 0:sz], W(head["ver"], k=VIN),
                                 xt[:, 0:sz], start=True, stop=True)
                nc.scalar.activation(vstream[:, off:off + sz], pp[:, 0:sz],
                                     AF.Identity, bias=Bv(head["ver_b"]))

            # ---------------- blocks ----------------
            infl, aggs, defl = sched["infl"], sched["agg"], sched["defl"]
            cnt_v, cnt_h = sched["cnt_v"], sched["cnt_h"]

            def gather_into(ps, half, wtile_idx, srcs, src_buf, sz=BC):
                """accumulate sum_k W[wtile_idx[k]].T @ src_buf[:, srcs[k]] into
                psum half."""
                o0 = half * BC
                if not srcs:
                    nc.tensor.matmul(ps[:, o0:o0 + sz], W(head["Wzero"]),
                                     src_buf[:, 0:sz], start=True, stop=True)
                    return
                for k, (wi, t) in enumerate(zip(wtile_idx, srcs)):
                    nc.tensor.matmul(ps[:, o0:o0 + sz], W(wi),
                                     src_buf[:, t * BC:t * BC + sz],
                                     start=(k == 0), stop=(k == len(srcs) - 1))

            for li in range(L):
                blk = blocks[li]

                # --- inflate: v1 = v0 + inf(h0) + inf_b ---
                for pr in range(NQ // 2):
                    ps = psA.tile([128, CH], f32, tag="ps")
                    for half in (0, 1):
                        n = 2 * pr + half
                        sl = infl[n]
                        gather_into(ps, half, [blk["inf"][s] for s, _ in sl],
                                    [t for _, t in sl], hstream)
                    o = pr * CH
                    nc.vector.scalar_tensor_tensor(
                        out=vstream[:, o:o + CH], in0=ps, scalar=Bv(blk["inf_b"]),
                        in1=vstream[:, o:o + CH], op0=OP.add, op1=OP.add)

                # --- agg: fused message gather, weights pre-scaled 1/c ---
                for pr in range(NQ // 2):
                    ps = psA.tile([128, CH], f32, tag="ps")
                    for half in (0, 1):
                        n = 2 * pr + half
                        gather_into(ps, half, [blk["msg"][cnt_v[n]]] * len(aggs[n]),
                                    aggs[n], vstream)
                    n0 = 2 * pr
                    if cnt_v[n0] == cnt_v[n0 + 1]:
                        nc.scalar.activation(
                            astream[:, n0 * BC:(n0 + 2) * BC], ps,
                            AF.Identity, bias=Bv(blk["agg_b"][cnt_v[n0]]))
                    else:
                        for half in (0, 1):
                            n = n0 + half
                            nc.scalar.activation(
                                astream[:, n * BC:(n + 1) * BC],
                                ps[:, half * BC:(half + 1) * BC],
                                AF.Identity, bias=Bv(blk["agg_b"][cnt_v[n]]))

                # --- upd + x + LN_v (Ln/Exp table phase) ---
                for off, sz in _chunks(WV):
                    pm = psA.tile([128, CH], f32, tag="ps")
                    nc.tensor.matmul(pm[:, 0:sz], W(blk["upd"][0]),
                                     vstream[:, off:off + sz], start=True, stop=False)
                    nc.tensor.matmul(pm[:, 0:sz], W(blk["upd"][1]),
                                     astream[:, off:off + sz], start=False, stop=True)
                    # x = v1 + mp + upd_b   (in place over v1)
                    nc.vector.scalar_tensor_tensor(
                        out=vstream[:, off:off + sz], in0=pm[:, 0:sz],
                        scalar=Bv(blk["upd_b"]), in1=vstream[:, off:off + sz],
                        op0=OP.add, op1=OP.add)
                    # LN: mean broadcast
                    pP = psA.tile([128, CH], f32, tag="ps")
                    nc.tensor.matmul(pP[:, 0:sz], W(head["Jones"]),
                                     vstream[:, off:off + sz], start=True, stop=True)
                    nc.vector.tensor_sub(vstream[:, off:off + sz],
                                         vstream[:, off:off + sz], pP[:, 0:sz])
                    sq = ring2.tile([128, CH], f16, tag="sq")
                    nc.scalar.activation(sq[:, 0:sz], vstream[:, off:off + sz],
                                         AF.Square, bias=Bv(head["zero_b"]))
                    pQ = psA.tile([128, CH], f32, tag="ps")
                    nc.tensor.matmul(pQ[:, 0:sz], W(head["Jones"]),
                                     sq[:, 0:sz], start=True, stop=True)
                    lnt = lring.tile([128, CH], f32, tag="lnt")
                    nc.scalar.activation(lnt[:, 0:sz], pQ[:, 0:sz], AF.Ln,
                                         bias=Bv(head["eps_b"]))
                    rr = ring2.tile([128, CH], f16, tag="rr")
                    nc.scalar.activation(rr[:, 0:sz], lnt[:, 0:sz], AF.Exp,
                                         bias=Bv(head["zero_b"]), scale=-0.5)
                    nc.vector.tensor_mul(vstream[:, off:off + sz],
                                         vstream[:, off:off + sz], rr[:, 0:sz])

                # --- vf FFN (Gelu table phase): v2 = g*zhat + (vf2(...)+vnb+vf2b)
                for off, sz in _chunks(WV):
                    g1 = ring2.tile([128, 2 * CH], f16, tag="g1")
                    pf2 = psB.tile([128, CH], f32, tag="psf")
                    for m in (0, 1):
                        pf1 = psA.tile([128, CH], f32, tag="ps")
                        nc.tensor.matmul(pf1[:, 0:sz], W(blk["vf1"][m]),
                                         vstream[:, off:off + sz],
                                         start=True, stop=True)
                        nc.scalar.activation(g1[:, m * CH:m * CH + sz],
                                             pf1[:, 0:sz], AF.Gelu,
                                             bias=Bv(blk["vf1_b"][m]))
                        nc.tensor.matmul(pf2[:, 0:sz], W(blk["vf2"][m]),
                                         g1[:, m * CH:m * CH + sz],
                                         start=(m == 0), stop=(m == 1))
                    tres = ring2.tile([128, CH], f16, tag="tres")
                    nc.scalar.activation(tres[:, 0:sz], pf2[:, 0:sz],
                                         AF.Identity, bias=Bv(blk["v2_b"]))
                    nc.vector.scalar_tensor_tensor(
                        out=vstream[:, off:off + sz], in0=vstream[:, off:off + sz],
                        scalar=Bv(blk["vn_g"]), in1=tres[:, 0:sz],
                        op0=OP.mult, op1=OP.add)

                # --- deflate: xh = h0 + def(v2) + def_b  (in place on hstream)
                for pr in range((TQ + 1) // 2):
                    n0 = 2 * pr
                    nhalf = min(2, TQ - n0)
                    ps = psA.tile([128, CH], f32, tag="ps")
                    for half in range(nhalf):
                        t = n0 + half
                        c = cnt_h[t]
                        gather_into(ps, half, [blk["def"][c]] * len(defl[t]),
                                    defl[t], vstream)
                    o = pr * CH
                    sz = nhalf * BC
                    nc.vector.scalar_tensor_tensor(
                        out=hstream[:, o:o + sz], in0=ps[:, 0:sz],
                        scalar=Bv(blk["def_b"]), in1=hstream[:, o:o + sz],
                        op0=OP.add, op1=OP.add)

                # --- LN_h (Ln/Exp phase) ---
                for off, sz in _chunks(WH):
                    pP = psA.tile([128, CH], f32, tag="ps")
                    nc.tensor.matmul(pP[:, 0:sz], W(head["Jones"]),
                                     hstream[:, off:off + sz], start=True, stop=True)
                    nc.vector.tensor_sub(hstream[:, off:off + sz],
                                         hstream[:, off:off + sz], pP[:, 0:sz])
                    sq = ring2.tile([128, CH], f16, tag="sq")
                    nc.scalar.activation(sq[:, 0:sz], hstream[:, off:off + sz],
                                         AF.Square, bias=Bv(head["zero_b"]))
                    pQ = psA.tile([128, CH], f32, tag="ps")
                    nc.tensor.matmul(pQ[:, 0:sz], W(head["Jones"]),
                                     sq[:, 0:sz], start=True, stop=True)
                    lnt = lring.tile([128, CH], f32, tag="lnt")
                    nc.scalar.activation(lnt[:, 0:sz], pQ[:, 0:sz], AF.Ln,
                                         bias=Bv(head["eps_b"]))
                    rr = ring2.tile([128, CH], f16, tag="rr")
                    nc.scalar.activation(rr[:, 0:sz], lnt[:, 0:sz], AF.Exp,
                                         bias=Bv(head["zero_b"]), scale=-0.5)
                    nc.vector.tensor_mul(hstream[:, off:off + sz],
                                         hstream[:, off:off + sz], rr[:, 0:sz])

                # --- hf FFN (Gelu phase) ---
                for off, sz in _chunks(WH):
                    g1 = ring2.tile([128, 2 * CH], f16, tag="g1")
                    pf2 = psB.tile([128, CH], f32, tag="psf")
                    for m in (0, 1):
                        pf1 = psA.tile([128, CH], f32, tag="ps")
                        nc.tensor.matmul(pf1[:, 0:sz], W(blk["hf1"][m]),
                                         hstream[:, off:off + sz],
                                         start=True, stop=True)
                        nc.scalar.activation(g1[:, m * CH:m * CH + sz],
                                             pf1[:, 0:sz], AF.Gelu,
                                             bias=Bv(blk["hf1_b"][m]))
                        nc.tensor.matmul(pf2[:, 0:sz], W(blk["hf2"][m]),
                                         g1[:, m * CH:m * CH + sz],
                                         start=(m == 0), stop=(m == 1))
                    tres = ring2.tile([128, CH], f16, tag="tres")
                    nc.scalar.activation(tres[:, 0:sz], pf2[:, 0:sz],
                                         AF.Identity, bias=Bv(blk["h1_b"]))
                    nc.vector.scalar_tensor_tensor(
                        out=hstream[:, off:off + sz], in0=hstream[:, off:off + sz],
                        scalar=Bv(blk["hn_g"]), in1=tres[:, 0:sz],
                        op0=OP.mult, op1=OP.add)

            # ---------------- head: pooling + out MLP ----------------
            hsum = psB.tile([128, BC], f32, tag="psf")
            for t in range(TQ):
                nc.tensor.matmul(hsum, W(head["I128"]),
                                 hstream[:, t * BC:(t + 1) * BC],
                                 start=(t == 0), stop=(t == TQ - 1))
            hp = ring.tile([128, BC], f16, tag="pool")
            nc.scalar.copy(hp, hsum)
            vsum = psB.tile([128, BC], f32, tag="psf")
            for t in range(NQ):
                nc.tensor.matmul(vsum, W(head["I128"]),
                                 vstream[:, t * BC:(t + 1) * BC],
                                 start=(t == 0), stop=(t == NQ - 1))
            vp = ring.tile([128, BC], f16, tag="pool")
            nc.scalar.copy(vp, vsum)

            qt = ring.tile([128, 2 * BC], f16, tag="qt")
            for m in (0, 1):
                po1 = psA.tile([128, BC], f32, tag="ps")
                nc.tensor.matmul(po1, W(head["o1"][0][m]), hp, start=True, stop=False)
                nc.tensor.matmul(po1, W(head["o1"][1][m]), vp, start=False, stop=False)
                nc.tensor.matmul(po1, W(head["o1"][2][m], k=64), ep_sb,
                                 start=False, stop=False)
                nc.tensor.matmul(po1, W(head["o1"][3][m]), pp_sb,
                                 start=False, stop=True)
                nc.scalar.activation(qt[:, m * BC:(m + 1) * BC], po1, AF.Gelu,
                                     bias=Bv(head["o1_b"][m]))
            for m in (0, 1):
                po2 = psA.tile([128, BC], f32, tag="ps")
                nc.tensor.matmul(po2, W(head["o2"][m][0]), qt[:, 0:BC],
                                 start=True, stop=False)
                nc.tensor.matmul(po2, W(head["o2"][m][1]), qt[:, BC:2 * BC],
                                 start=False, stop=True)
                osb = ring.tile([128, BC], f32, tag="osb")
                nc.scalar.activation(osb, po2, AF.Identity,
                                     bias=Bv(head["o2_b"][m]))
                nc.sync.dma_start(out=d_out[m * 128:(m + 1) * 128, :], in_=osb)

    bacc.get_activation_tables = _gat
    try:
        nc.compile()
    finally:
        bacc.get_activation_tables = _orig_gat
    return nc


def _get_program(params, vertex_to_hex, hex_to_vertex, vertex_adj):
    key = (np.asarray(vertex_to_hex).tobytes(),
           np.asarray(hex_to_vertex).tobytes(),
           np.asarray(vertex_adj).tobytes())
    if key in _CACHE:
        return _CACHE[key]
    wpack, bpack, blocks, head, sched, pv, ph = _prep(
        params, vertex_to_hex, hex_to_vertex, vertex_adj)
    nc = _build(wpack.shape[1], bpack.shape[1], blocks, head, sched)
    _CACHE[key] = (nc, wpack, bpack, pv, ph)
    return _CACHE[key]


def kernel(hex_features, vertex_features, edge_features, player_features,
           params, vertex_to_hex, hex_to_vertex, vertex_adj):
    from concourse.bass_utils import run_bass_kernel_spmd

    nc, wpack, bpack, pv, ph = _get_program(
        params, vertex_to_hex, hex_to_vertex, vertex_adj)

    hexf = np.asarray(hex_features, np.float32)
    verf = np.asarray(vertex_features, np.float32)
    edgf = np.asarray(edge_features, np.float32)
    plyf = np.asarray(player_features, np.float32)

    in_maps = []
    for c in range(N_CORES):
        sl = slice(c * BC, (c + 1) * BC)
        hT = hexf[sl][:, ph, :].transpose(2, 1, 0).reshape(HIN, WH)
        vT = verf[sl][:, pv, :].transpose(2, 1, 0).reshape(VIN, WV)
        eT = edgf[sl].transpose(2, 1, 0).reshape(EIN, WE)
        pT = plyf[sl].T
        in_maps.append({
            "wpack": wpack,
            "bpack": bpack,
            "hexT": np.ascontiguousarray(hT, dtype=np.float16),
            "vertT": np.ascontiguousarray(vT, dtype=np.float16),
            "edgeT": np.ascontiguousarray(eT, dtype=np.float16),
            "playerT": np.ascontiguousarray(pT, dtype=np.float16),
        })

    res = run_bass_kernel_spmd(nc, in_maps, core_ids=list(range(N_CORES)))
    out = np.concatenate([r["outT"].T for r in res.results], axis=0)
    return out.astype(np.float32)
